# revision 14
# baseline (speedup 1.0000x reference)
"""HQLinear (VQ codebook) Trainium2 kernel — data-parallel tokens, int8 x,
sharded dequantize with on-device AllGather of the weight.

Computes: out = einsum('bsi,oi->bso', x, codebook[indices].reshape(O, I) * scales)
on 8 NeuronCores. The axon tunnel moves ~55-90 MB/s, so wall clock is
dominated by host->device bytes. Wire layout (~100 MB total vs ~1.6 GB for
the naive tensor-parallel layout):
  - x sharded over tokens (512/core), int8-quantized per token on host
    (45 MB); the per-token scale is applied in the f32 epilogue, so the
    int8 values flow exactly through the bf16 matmul.
  - indices sharded over out rows (512 rows/core, compact int16, 11 MB
    total): each core gathers + transposes its 1/8 of the weight, then a
    DRAM AllGather replicates the pair-packed transposed weight (the 8x
    expansion rides NeuronLink instead of the tunnel).
  - codebook uploaded pre-cast bf16 (0.5 MB/core).
  - output int8-quantized on device per (o-tile, token) block with f32
    block scales (down 17 MB, donated zeros 17 MB); host dequantizes.

Per-core pipeline:
  - codebook bf16 copied DRAM->DRAM into 256B-stride rows (dma_gather's
    source stride must be a multiple of 256B).
  - x shard [512 tok, 11008] int8 loaded in 2048-column chunks, cast
    i8->bf16 (exact), PE-transposed pair-packed (2 bf16 viewed as one f32
    lane) into a resident SBUF xT (i on partitions), 11.3 MB.
  - per local 128-out-row tile (4): DMA-gather 1376*128 codebook vectors
    (16B each) into SBUF staging [128 o, 11008 i] bf16, PE-transpose
    pair-packed to wT [i-pairs, 128 o], store to DRAM wt_loc.
  - AllGather wt_loc [4,128,5504] f32 -> wt_all [32,128,5504] (ranks
    concatenate: global o-tile ot <- core ot//4, local tile ot%4).
  - per global o-tile (32): DMA wt_all[ot] to SBUF, 86 bf16 matmuls
    (N=512 tokens) accumulate into PSUM [128 o, 512 t]; epilogue applies
    scales[o] (per-partition) then the per-token x scale (free-dim
    broadcast) in f32, computes the per-token block absmax with a gpsimd
    partition reduce, and emits int8 values + f32 scales.
  - index lists are uploaded compact ([16, 688] per gather) and replicated
    to the 8 16-row partition groups by a broadcast DMA on device.

Pair packing: an f32 lane at pair index f holds bf16 values for i = 2f,
2f+1; matmul (icp, h) contracts partitions p <-> i = 128*2*icp + 2p + h on
both operands via stride-2 bf16 views.
"""

from contextlib import ExitStack

import numpy as np

import concourse.ap_utils as ap_utils
import concourse.bass as bass
import concourse.tile as tile
from concourse import bacc, mybir
from concourse.bass import ts, ds, exact_div
from concourse.bass_isa import ReduceOp
from concourse.masks import make_identity
import concourse.bass_utils as bass_utils

F32 = mybir.dt.float32
BF16 = mybir.dt.bfloat16
I16 = mybir.dt.int16
I8 = mybir.dt.int8
P = 128

N_CORES = 8
OUT_F = 4096
IN_F = 11008
VDIM = 8
N_CODES = 32768
BATCH, SEQ = 2, 2048
T = BATCH * SEQ            # 4096 tokens
TSH = T // N_CORES         # 512 tokens per core
T_TILES = TSH // P         # 4 token tiles per core
O_TILES = OUT_F // P       # 32 out-row tiles (full weight, post-allgather)
LOT = O_TILES // N_CORES   # 4 local o-tiles gathered per core
NJ = IN_F // VDIM          # 1376 index columns per out row
JC = 16                    # gather chunks per 128-row o-tile
NJC = NJ // JC             # 86 j-columns per gather (11008 idx <= HW limit)
ICP = IN_F // 256          # 43 pair chunks (256 i-values each)
GRP = 8                    # icp per transpose/copy group (2 PSUM banks)

NP_BF16 = mybir.dt.np(BF16)


def _dma_gather_small(gp, out_ap, in_ap, idxs_ap, num_idxs, elem_size, elem_step):
    """dma_gather with small elements (16B); source stride still 256B-aligned.

    Vector g comes from in_[list[g], :elem_size] (row stride elem_step) and
    lands at out[g%128, g//128, :]. Index list int16, wrapped: idxs[c, s] =
    list[s*16 + c] for c in 0..15, replicated across the 8 16-row groups.
    """
    assert idxs_ap.dtype == I16
    assert in_ap.dtype == out_ap.dtype
    assert in_ap.space == bass.MemorySpace.DRAM
    assert idxs_ap.space == bass.MemorySpace.SBUF
    assert out_ap.space == bass.MemorySpace.SBUF
    assert ap_utils.ap_is_contiguous(in_ap.ap[1:])
    assert ap_utils.ap_is_contiguous(out_ap.ap[1:])
    assert ap_utils.ap_is_contiguous(idxs_ap.ap[1:])
    assert in_ap.ap[-1][1] == elem_size
    assert out_ap.ap[-1][1] == elem_size
    assert in_ap.ap[0][0] == elem_step
    stride_bytes_256 = exact_div(elem_step * mybir.dt.size(in_ap.dtype), 256)
    assert 0 < stride_bytes_256 < 256
    _in_ap = gp.lower_ap_dma(in_ap, for_custom_bir_dma=True)
    _idxs_ap = gp.lower_ap(idxs_ap)
    _out_ap = gp.lower_ap(out_ap)
    return gp.add_instruction(
        mybir.InstDMAGatherAnt(
            name=gp.bass.get_next_instruction_name(),
            ins=[*_in_ap, _idxs_ap, gp.lower_val_access(gp.to_reg(num_idxs))],
            outs=[_out_ap],
            transpose=False,
            num_idxs=num_idxs,
            elem_size=elem_size,
            stride_bytes_256=stride_bytes_256,
            gen_mode=0,
            single_packet=False,
            queue_num=0,
            sbuf_tokens_per_rank=0,
            sbuf_free_dim_per_rank=0,
            sbuf_free_dim_pad_per_rank=0,
            sbuf_byte_offset=0,
        )
    )


def build():
    """Build and compile the per-core kernel. Returns the Bacc instance."""
    groups = [(g, min(GRP, ICP - g)) for g in range(0, ICP, GRP)]

    nc = bacc.Bacc("TRN2", target_bir_lowering=False, debug=False,
                   enable_asserts=False, num_devices=N_CORES)

    xq = nc.dram_tensor("xq", [TSH, IN_F], I8, kind="ExternalInput").ap()
    xsc = nc.dram_tensor("xsc", [1, TSH], F32, kind="ExternalInput").ap()
    cbb = nc.dram_tensor("cbb", [N_CODES, VDIM], BF16, kind="ExternalInput").ap()
    idx16 = nc.dram_tensor("idx16", [LOT * JC, 16, NJC * VDIM], I16,
                           kind="ExternalInput").ap()
    # scales pre-transposed on host: sc_t[p, ot] = scales[ot*128 + p]
    scales_t = nc.dram_tensor("scales_t", [P, O_TILES], F32,
                              kind="ExternalInput").ap()
    out = nc.dram_tensor("out", [OUT_F, TSH], I8, kind="ExternalOutput").ap()
    out_s = nc.dram_tensor("out_s", [O_TILES, TSH], F32, kind="ExternalOutput").ap()
    cb_pad = nc.dram_tensor("cb_pad", [N_CODES, 128], BF16, kind="Internal").ap()
    wt_loc = nc.dram_tensor("wt_loc", [LOT, P, ICP * P], F32, kind="Internal").ap()
    wt_all = nc.dram_tensor("wt_all", [O_TILES, P, ICP * P], F32,
                            kind="Internal", addr_space="Shared").ap()

    with tile.TileContext(nc) as tc, ExitStack() as ctx:
        const_pool = ctx.enter_context(tc.tile_pool(name="const", bufs=1))
        xt_pool = ctx.enter_context(tc.tile_pool(name="xt", bufs=1))

        identity = const_pool.tile([P, P], F32)
        make_identity(nc, identity[:])

        sc_sb = const_pool.tile([P, O_TILES], F32)
        nc.sync.dma_start(sc_sb[:], scales_t)

        # per-token x scale, broadcast across partitions: [128, 512]
        sxt_sb = const_pool.tile([P, TSH], F32)
        nc.sync.dma_start(sxt_sb[:], xsc.to_broadcast([P, TSH]))

        # --- codebook bf16 -> padded 256B-stride rows (DRAM->DRAM) ---
        cb_pad3 = cb_pad.rearrange("(p r) c -> p r c", p=P)[:, :, :VDIM]
        nc.sync.dma_start(
            cb_pad3, cbb.rearrange("(p r) c -> p r c", p=P))

        # resident xT (pair-packed, f32-typed): free = icp*TSH + tt*128 + t
        xT = xt_pool.tile([P, ICP * TSH], F32)
        xT4 = xT[:].rearrange("p (i tt t) -> p i tt t", tt=T_TILES, t=P)

        stage_pool = ctx.enter_context(tc.tile_pool(name="stage", bufs=2))
        tpsum_pool = ctx.enter_context(
            tc.tile_pool(name="tpsum", bufs=2, space="PSUM"))
        wt_pool = ctx.enter_context(tc.tile_pool(name="wt", bufs=2))
        idx_pool = ctx.enter_context(tc.tile_pool(name="idxp", bufs=3))
        xq_pool = ctx.enter_context(tc.tile_pool(name="xq", bufs=2))
        xs_pool = ctx.enter_context(tc.tile_pool(name="xs", bufs=2))

        # --- dequant x (i8 -> bf16, exact) + transpose into resident xT ---
        for tt in range(T_TILES):
            for g0, glen in groups:
                ncol = glen * 256
                xq_t = xq_pool.tile([P, GRP * 128], BF16, tag="xq")
                nc.sync.dma_start(xq_t[:].bitcast(I8)[:, :ncol],
                                  xq[ts(tt, P), g0 * 256:(g0 + glen) * 256])
                xs = xs_pool.tile([P, GRP * 256], BF16, tag="xs")
                nc.vector.tensor_copy(xs[:, :ncol], xq_t[:].bitcast(I8)[:, :ncol])
                xsv = xs[:].bitcast(F32)  # [128 t, pair lanes]
                tp = tpsum_pool.tile([P, GRP * P], F32, tag="tp")
                for q in range(glen):
                    nc.tensor.transpose(
                        out=tp[:, ts(q, P)],
                        in_=xsv[:, ts(q, P)],
                        identity=identity[:],
                    )
                src = tp[:, :glen * P].rearrange("p (i t) -> p i t", t=P)
                nc.vector.tensor_copy(xT4[:, ds(g0, glen), tt, :], src)

        # bf16 view of xT: free = 2*(icp*TSH + tt*128 + t) + h
        xTb = xT[:].bitcast(BF16)

        # --- gather + transpose local weight shard, store to wt_loc ---
        for lot in range(LOT):
            wst = stage_pool.tile([P, IN_F], BF16, tag="stage")
            # gather: wst[p, 8j:8j+8] = cbb[idx[lot*128+p, j], :]
            for jc in range(JC):
                idx_t = idx_pool.tile([P, NJC * VDIM], I16, tag="idx")
                src = idx16[lot * JC + jc, :, :]
                nc.sync.dma_start(
                    idx_t[:], src.unsqueeze(0).to_broadcast(
                        [P // 16, 16, NJC * VDIM]))
                _dma_gather_small(
                    nc.gpsimd,
                    out_ap=wst[:, jc * NJC * VDIM:(jc + 1) * NJC * VDIM]
                        .rearrange("p (n e) -> p n e", e=VDIM),
                    in_ap=cb_pad[:, :VDIM],
                    idxs_ap=idx_t[:],
                    num_idxs=NJC * P,
                    elem_size=VDIM,
                    elem_step=128,
                )
            # transpose to wT [i-pairs, 128 o] (f32 pair lanes)
            wT = wt_pool.tile([P, ICP * P], F32, tag="wt")
            wT3 = wT[:].rearrange("p (i o) -> p i o", o=P)
            wstv = wst[:].bitcast(F32)  # [128 o, 5504 pair lanes]
            for g0, glen in groups:
                tp = tpsum_pool.tile([P, GRP * P], F32, tag="tp")
                for q in range(glen):
                    nc.tensor.transpose(
                        out=tp[:, ts(q, P)],
                        in_=wstv[:, ts(g0 + q, P)],
                        identity=identity[:],
                    )
                src = tp[:, :glen * P].rearrange("p (i o) -> p i o", o=P)
                nc.vector.tensor_copy(wT3[:, ds(g0, glen), :], src)
            nc.sync.dma_start(wt_loc[lot], wT[:])

        # --- allgather the pair-packed transposed weight across cores ---
        nc.gpsimd.collective_compute(
            "AllGather",
            mybir.AluOpType.bypass,
            replica_groups=[list(range(N_CORES))],
            ins=[wt_loc[:].opt()],
            outs=[wt_all[:].opt()],
        )

        # --- main loop over global out-row tiles ---
        opsum_pool = ctx.enter_context(
            tc.tile_pool(name="opsum", bufs=2, space="PSUM"))
        tmp_pool = ctx.enter_context(tc.tile_pool(name="tmp", bufs=2))
        am_pool = ctx.enter_context(tc.tile_pool(name="am", bufs=2))
        qs_pool = ctx.enter_context(tc.tile_pool(name="qs", bufs=2))
        qi_pool = ctx.enter_context(tc.tile_pool(name="qi", bufs=2))

        for ot in range(O_TILES):
            wT = wt_pool.tile([P, ICP * P], F32, tag="wt")
            nc.sync.dma_start(wT[:], wt_all[ot])
            wTb = wT[:].bitcast(BF16)  # free = 2*(icp*128 + o) + h

            # 86 matmuls accumulate PSUM [128 o, 512 t]
            po = opsum_pool.tile([P, TSH], F32, tag="op")
            for icp in range(ICP):
                for h in range(2):
                    lhsT = wTb[:, 2 * icp * P + h: 2 * (icp + 1) * P: 2]
                    rhs = xTb[:, 2 * icp * TSH + h: 2 * (icp + 1) * TSH: 2]
                    nc.tensor.matmul(out=po[:], lhsT=lhsT, rhs=rhs,
                                     start=(icp == 0 and h == 0),
                                     stop=(icp == ICP - 1 and h == 1))

            # epilogue: scales[o] (per-partition), then per-token x scale
            # (free-dim broadcast); int8-quantize per (o-tile, token) block
            tmp = tmp_pool.tile([P, TSH], F32, tag="tmp")
            nc.vector.tensor_scalar(
                out=tmp[:], in0=po[:], scalar1=sc_sb[:, ot:ot + 1],
                scalar2=None, op0=mybir.AluOpType.mult)
            nc.vector.tensor_tensor(
                out=tmp[:], in0=tmp[:], in1=sxt_sb[:],
                op=mybir.AluOpType.mult)
            am = am_pool.tile([P, TSH], F32, tag="am")
            nc.gpsimd.partition_all_reduce(am[:], tmp[:], P, ReduceOp.absmax)
            qs = qs_pool.tile([P, TSH], F32, tag="qs")
            nc.vector.tensor_scalar(
                out=qs[:], in0=am[:], scalar1=1.0 / 127.0,
                scalar2=None, op0=mybir.AluOpType.mult)
            nc.vector.reciprocal(am[:], qs[:])   # am <- 127/absmax
            nc.vector.tensor_tensor(
                out=tmp[:], in0=tmp[:], in1=am[:],
                op=mybir.AluOpType.mult)
            qi = qi_pool.tile([P, TSH], I8, tag="qi")
            nc.vector.tensor_copy(qi[:], tmp[:])
            nc.sync.dma_start(out[ts(ot, P), :], qi[:])
            nc.sync.dma_start(out_s[ot:ot + 1, :], qs[0:1, :])

    nc.compile()
    return nc


def prep_idx16(idx2):
    """Host prep: full [OUT_F, NJ] int32 -> compact wrapped int16 gather
    lists [O_TILES*JC, 16, NJC*VDIM] (no 8x replication; device broadcasts).

    Per (ot, jc): glist[g] for g = j*128 + o, wrapped[c, s] = glist[s*16+c].
    """
    A = idx2.reshape(O_TILES, P, JC, NJC)
    B = A.transpose(0, 2, 3, 1)                 # [ot, jc, j, o]
    C = B.reshape(O_TILES, JC, NJC * P)         # glist, g = j*128 + o
    D = C.reshape(O_TILES, JC, (NJC * P) // 16, 16)
    E = D.transpose(0, 1, 3, 2)                 # [ot, jc, c, s]
    return np.ascontiguousarray(E.reshape(O_TILES * JC, 16, NJC * VDIM)).astype(np.int16)


_NC_CACHE = []


def _get_nc():
    if not _NC_CACHE:
        _NC_CACHE.append(build())
    return _NC_CACHE[0]


def make_in_maps(x, indices, codebook, scales):
    x2 = np.asarray(x).reshape(T, IN_F)
    amax = np.abs(x2).max(axis=1, keepdims=True)
    s = np.where(amax > 0, amax / 127.0, 1.0).astype(np.float32)
    xq = np.clip(np.rint(x2 * (1.0 / s)), -127, 127).astype(np.int8)
    sflat = s.reshape(T)
    idx2 = np.asarray(indices, dtype=np.int32).reshape(OUT_F, NJ)
    idx16 = prep_idx16(idx2)
    sc_t = np.ascontiguousarray(
        np.asarray(scales, dtype=np.float32).reshape(O_TILES, P).T)
    cbv = np.asarray(codebook, dtype=np.float32).astype(NP_BF16)
    in_maps = []
    for c in range(N_CORES):
        in_maps.append({
            "xq": xq[c * TSH:(c + 1) * TSH],
            "xsc": np.ascontiguousarray(
                sflat[c * TSH:(c + 1) * TSH]).reshape(1, TSH),
            "cbb": cbv,
            "idx16": idx16[c * LOT * JC:(c + 1) * LOT * JC],
            "scales_t": sc_t,
        })
    return in_maps


def assemble_out(res):
    """int8 [4096 o, 512 t] + scales [32, 512] per core -> [BATCH, SEQ, OUT_F] f32."""
    out = np.empty((T, OUT_F), dtype=np.float32)
    for c in range(N_CORES):
        q = res.results[c]["out"].astype(np.float32).reshape(O_TILES, P, TSH)
        s = res.results[c]["out_s"]
        out[c * TSH:(c + 1) * TSH, :] = (q * s[:, None, :]).reshape(OUT_F, TSH).T
    return out.reshape(BATCH, SEQ, OUT_F)


def kernel(x, indices, codebook, scales):
    nc = _get_nc()
    in_maps = make_in_maps(x, indices, codebook, scales)
    res = bass_utils.run_bass_kernel_spmd(nc, in_maps, core_ids=list(range(N_CORES)))
    return assemble_out(res)


# revision 15
# speedup vs baseline: 1.1106x; 1.1106x over previous
"""HQLinear (VQ codebook) Trainium2 kernel — data-parallel tokens, int8 x,
sharded dequantize with on-device AllGather of the weight.

Computes: out = einsum('bsi,oi->bso', x, codebook[indices].reshape(O, I) * scales)
on 8 NeuronCores. The axon tunnel moves ~55-90 MB/s, so wall clock is
dominated by host->device bytes. Wire layout (~100 MB total vs ~1.6 GB for
the naive tensor-parallel layout):
  - x sharded over tokens (512/core), int8-quantized per token on host
    (45 MB); the per-token scale is applied in the f32 epilogue, so the
    int8 values flow exactly through the bf16 matmul.
  - indices sharded over out rows (512 rows/core, compact int16, 11 MB
    total): each core gathers + transposes its 1/8 of the weight, then a
    DRAM AllGather replicates the pair-packed transposed weight (the 8x
    expansion rides NeuronLink instead of the tunnel).
  - codebook uploaded pre-cast bf16 (0.5 MB/core).
  - output int8-quantized on device per (o-tile, token) block with f32
    block scales (down 17 MB, donated zeros 17 MB); host dequantizes.

Per-core pipeline:
  - codebook bf16 copied DRAM->DRAM into 256B-stride rows (dma_gather's
    source stride must be a multiple of 256B).
  - x shard [512 tok, 11008] int8 loaded in 2048-column chunks, cast
    i8->bf16 (exact), PE-transposed pair-packed (2 bf16 viewed as one f32
    lane) into a resident SBUF xT (i on partitions), 11.3 MB.
  - per local 128-out-row tile (4): DMA-gather 1376*128 codebook vectors
    (16B each) into SBUF staging [128 o, 11008 i] bf16, PE-transpose
    pair-packed to wT [i-pairs, 128 o], store to DRAM wt_loc.
  - AllGather wt_loc [4,128,5504] f32 -> wt_all [32,128,5504] (ranks
    concatenate: global o-tile ot <- core ot//4, local tile ot%4).
  - per global o-tile (32): DMA wt_all[ot] to SBUF, 86 bf16 matmuls
    (N=512 tokens) accumulate into PSUM [128 o, 512 t]; epilogue applies
    scales[o] (per-partition) then the per-token x scale (free-dim
    broadcast) in f32, computes the per-token block absmax with a gpsimd
    partition reduce, and emits int8 values + f32 scales.
  - index lists are uploaded compact ([16, 688] per gather) and replicated
    to the 8 16-row partition groups by a broadcast DMA on device.

Pair packing: an f32 lane at pair index f holds bf16 values for i = 2f,
2f+1; matmul (icp, h) contracts partitions p <-> i = 128*2*icp + 2p + h on
both operands via stride-2 bf16 views.
"""

from contextlib import ExitStack

import numpy as np

import concourse.ap_utils as ap_utils
import concourse.bass as bass
import concourse.tile as tile
from concourse import bacc, mybir
from concourse.bass import ts, ds, exact_div
from concourse.bass_isa import ReduceOp
from concourse.masks import make_identity
import concourse.bass_utils as bass_utils

F32 = mybir.dt.float32
BF16 = mybir.dt.bfloat16
I16 = mybir.dt.int16
I8 = mybir.dt.int8
P = 128

N_CORES = 8
OUT_F = 4096
IN_F = 11008
VDIM = 8
N_CODES = 32768
BATCH, SEQ = 2, 2048
T = BATCH * SEQ            # 4096 tokens
TSH = T // N_CORES         # 512 tokens per core
T_TILES = TSH // P         # 4 token tiles per core
O_TILES = OUT_F // P       # 32 out-row tiles (full weight, post-allgather)
LOT = O_TILES // N_CORES   # 4 local o-tiles gathered per core
NJ = IN_F // VDIM          # 1376 index columns per out row
JC = 16                    # gather chunks per 128-row o-tile
NJC = NJ // JC             # 86 j-columns per gather (11008 idx <= HW limit)
ICP = IN_F // 256          # 43 pair chunks (256 i-values each)
GRP = 8                    # icp per transpose/copy group (2 PSUM banks)

NP_BF16 = mybir.dt.np(BF16)


def _dma_gather_small(gp, out_ap, in_ap, idxs_ap, num_idxs, elem_size, elem_step):
    """dma_gather with small elements (16B); source stride still 256B-aligned.

    Vector g comes from in_[list[g], :elem_size] (row stride elem_step) and
    lands at out[g%128, g//128, :]. Index list int16, wrapped: idxs[c, s] =
    list[s*16 + c] for c in 0..15, replicated across the 8 16-row groups.
    """
    assert idxs_ap.dtype == I16
    assert in_ap.dtype == out_ap.dtype
    assert in_ap.space == bass.MemorySpace.DRAM
    assert idxs_ap.space == bass.MemorySpace.SBUF
    assert out_ap.space == bass.MemorySpace.SBUF
    assert ap_utils.ap_is_contiguous(in_ap.ap[1:])
    assert ap_utils.ap_is_contiguous(out_ap.ap[1:])
    assert ap_utils.ap_is_contiguous(idxs_ap.ap[1:])
    assert in_ap.ap[-1][1] == elem_size
    assert out_ap.ap[-1][1] == elem_size
    assert in_ap.ap[0][0] == elem_step
    stride_bytes_256 = exact_div(elem_step * mybir.dt.size(in_ap.dtype), 256)
    assert 0 < stride_bytes_256 < 256
    _in_ap = gp.lower_ap_dma(in_ap, for_custom_bir_dma=True)
    _idxs_ap = gp.lower_ap(idxs_ap)
    _out_ap = gp.lower_ap(out_ap)
    return gp.add_instruction(
        mybir.InstDMAGatherAnt(
            name=gp.bass.get_next_instruction_name(),
            ins=[*_in_ap, _idxs_ap, gp.lower_val_access(gp.to_reg(num_idxs))],
            outs=[_out_ap],
            transpose=False,
            num_idxs=num_idxs,
            elem_size=elem_size,
            stride_bytes_256=stride_bytes_256,
            gen_mode=0,
            single_packet=False,
            queue_num=0,
            sbuf_tokens_per_rank=0,
            sbuf_free_dim_per_rank=0,
            sbuf_free_dim_pad_per_rank=0,
            sbuf_byte_offset=0,
        )
    )


def build():
    """Build and compile the per-core kernel. Returns the Bacc instance."""
    groups = [(g, min(GRP, ICP - g)) for g in range(0, ICP, GRP)]

    nc = bacc.Bacc("TRN2", target_bir_lowering=False, debug=False,
                   enable_asserts=False, num_devices=N_CORES)

    xq = nc.dram_tensor("xq", [TSH, IN_F], I8, kind="ExternalInput").ap()
    xsc = nc.dram_tensor("xsc", [1, TSH], F32, kind="ExternalInput").ap()
    cbb = nc.dram_tensor("cbb", [N_CODES, VDIM], BF16, kind="ExternalInput").ap()
    idx16 = nc.dram_tensor("idx16", [LOT * JC, 16, NJC * VDIM], I16,
                           kind="ExternalInput").ap()
    # scales pre-transposed on host: sc_t[p, ot] = scales[ot*128 + p]
    scales_t = nc.dram_tensor("scales_t", [P, O_TILES], F32,
                              kind="ExternalInput").ap()
    out = nc.dram_tensor("out", [OUT_F, TSH], I8, kind="ExternalOutput").ap()
    out_s = nc.dram_tensor("out_s", [O_TILES, TSH], F32, kind="ExternalOutput").ap()
    cb_pad = nc.dram_tensor("cb_pad", [N_CODES, 128], BF16, kind="Internal").ap()
    wt_loc = nc.dram_tensor("wt_loc", [LOT, P, ICP * P], F32, kind="Internal").ap()
    wt_all = nc.dram_tensor("wt_all", [O_TILES, P, ICP * P], F32,
                            kind="Internal", addr_space="Shared").ap()

    with tile.TileContext(nc) as tc, ExitStack() as ctx:
        const_pool = ctx.enter_context(tc.tile_pool(name="const", bufs=1))
        xt_pool = ctx.enter_context(tc.tile_pool(name="xt", bufs=1))

        identity = const_pool.tile([P, P], F32)
        make_identity(nc, identity[:])

        sc_sb = const_pool.tile([P, O_TILES], F32)
        nc.sync.dma_start(sc_sb[:], scales_t)

        # per-token x scale, broadcast across partitions: [128, 512]
        sxt_sb = const_pool.tile([P, TSH], F32)
        nc.sync.dma_start(sxt_sb[:], xsc.to_broadcast([P, TSH]))

        # --- codebook bf16 -> padded 256B-stride rows (DRAM->DRAM) ---
        cb_pad3 = cb_pad.rearrange("(p r) c -> p r c", p=P)[:, :, :VDIM]
        nc.sync.dma_start(
            cb_pad3, cbb.rearrange("(p r) c -> p r c", p=P))

        # resident xT (pair-packed, f32-typed): free = icp*TSH + tt*128 + t
        xT = xt_pool.tile([P, ICP * TSH], F32)
        xT4 = xT[:].rearrange("p (i tt t) -> p i tt t", tt=T_TILES, t=P)

        stage_pool = ctx.enter_context(tc.tile_pool(name="stage", bufs=2))
        tpsum_pool = ctx.enter_context(
            tc.tile_pool(name="tpsum", bufs=2, space="PSUM"))
        wt_pool = ctx.enter_context(tc.tile_pool(name="wt", bufs=2))
        idx_pool = ctx.enter_context(tc.tile_pool(name="idxp", bufs=3))
        xq_pool = ctx.enter_context(tc.tile_pool(name="xq", bufs=2))
        xs_pool = ctx.enter_context(tc.tile_pool(name="xs", bufs=2))

        # --- gather + transpose local weight shard, store to wt_loc ---
        for lot in range(LOT):
            wst = stage_pool.tile([P, IN_F], BF16, tag="stage")
            # gather: wst[p, 8j:8j+8] = cbb[idx[lot*128+p, j], :]
            for jc in range(JC):
                idx_t = idx_pool.tile([P, NJC * VDIM], I16, tag="idx")
                src = idx16[lot * JC + jc, :, :]
                nc.sync.dma_start(
                    idx_t[:], src.unsqueeze(0).to_broadcast(
                        [P // 16, 16, NJC * VDIM]))
                _dma_gather_small(
                    nc.gpsimd,
                    out_ap=wst[:, jc * NJC * VDIM:(jc + 1) * NJC * VDIM]
                        .rearrange("p (n e) -> p n e", e=VDIM),
                    in_ap=cb_pad[:, :VDIM],
                    idxs_ap=idx_t[:],
                    num_idxs=NJC * P,
                    elem_size=VDIM,
                    elem_step=128,
                )
            # transpose to wT [i-pairs, 128 o] (f32 pair lanes)
            wT = wt_pool.tile([P, ICP * P], F32, tag="wt")
            wT3 = wT[:].rearrange("p (i o) -> p i o", o=P)
            wstv = wst[:].bitcast(F32)  # [128 o, 5504 pair lanes]
            for g0, glen in groups:
                tp = tpsum_pool.tile([P, GRP * P], F32, tag="tp")
                for q in range(glen):
                    nc.tensor.transpose(
                        out=tp[:, ts(q, P)],
                        in_=wstv[:, ts(g0 + q, P)],
                        identity=identity[:],
                    )
                src = tp[:, :glen * P].rearrange("p (i o) -> p i o", o=P)
                nc.vector.tensor_copy(wT3[:, ds(g0, glen), :], src)
            nc.sync.dma_start(wt_loc[lot], wT[:])

        # --- allgather the pair-packed transposed weight across cores ---
        nc.gpsimd.collective_compute(
            "AllGather",
            mybir.AluOpType.bypass,
            replica_groups=[list(range(N_CORES))],
            ins=[wt_loc[:].opt()],
            outs=[wt_all[:].opt()],
        )

        # --- dequant x (i8 -> bf16, exact) + transpose into resident xT ---
        for tt in range(T_TILES):
            for g0, glen in groups:
                ncol = glen * 256
                xq_t = xq_pool.tile([P, GRP * 128], BF16, tag="xq")
                nc.sync.dma_start(xq_t[:].bitcast(I8)[:, :ncol],
                                  xq[ts(tt, P), g0 * 256:(g0 + glen) * 256])
                xs = xs_pool.tile([P, GRP * 256], BF16, tag="xs")
                nc.vector.tensor_copy(xs[:, :ncol], xq_t[:].bitcast(I8)[:, :ncol])
                xsv = xs[:].bitcast(F32)  # [128 t, pair lanes]
                tp = tpsum_pool.tile([P, GRP * P], F32, tag="tp")
                for q in range(glen):
                    nc.tensor.transpose(
                        out=tp[:, ts(q, P)],
                        in_=xsv[:, ts(q, P)],
                        identity=identity[:],
                    )
                src = tp[:, :glen * P].rearrange("p (i t) -> p i t", t=P)
                nc.vector.tensor_copy(xT4[:, ds(g0, glen), tt, :], src)

        # bf16 view of xT: free = 2*(icp*TSH + tt*128 + t) + h
        xTb = xT[:].bitcast(BF16)

        # --- main loop over global out-row tiles ---
        opsum_pool = ctx.enter_context(
            tc.tile_pool(name="opsum", bufs=2, space="PSUM"))
        tmp_pool = ctx.enter_context(tc.tile_pool(name="tmp", bufs=2))
        am_pool = ctx.enter_context(tc.tile_pool(name="am", bufs=2))
        qs_pool = ctx.enter_context(tc.tile_pool(name="qs", bufs=2))
        qi_pool = ctx.enter_context(tc.tile_pool(name="qi", bufs=2))

        for ot in range(O_TILES):
            wT = wt_pool.tile([P, ICP * P], F32, tag="wt")
            nc.sync.dma_start(wT[:], wt_all[ot])
            wTb = wT[:].bitcast(BF16)  # free = 2*(icp*128 + o) + h

            # 86 matmuls accumulate PSUM [128 o, 512 t]
            po = opsum_pool.tile([P, TSH], F32, tag="op")
            for icp in range(ICP):
                for h in range(2):
                    lhsT = wTb[:, 2 * icp * P + h: 2 * (icp + 1) * P: 2]
                    rhs = xTb[:, 2 * icp * TSH + h: 2 * (icp + 1) * TSH: 2]
                    nc.tensor.matmul(out=po[:], lhsT=lhsT, rhs=rhs,
                                     start=(icp == 0 and h == 0),
                                     stop=(icp == ICP - 1 and h == 1))

            # epilogue: scales[o] (per-partition), then per-token x scale
            # (free-dim broadcast); int8-quantize per (o-tile, token) block
            tmp = tmp_pool.tile([P, TSH], F32, tag="tmp")
            nc.vector.tensor_scalar(
                out=tmp[:], in0=po[:], scalar1=sc_sb[:, ot:ot + 1],
                scalar2=None, op0=mybir.AluOpType.mult)
            nc.vector.tensor_tensor(
                out=tmp[:], in0=tmp[:], in1=sxt_sb[:],
                op=mybir.AluOpType.mult)
            am = am_pool.tile([P, TSH], F32, tag="am")
            nc.gpsimd.partition_all_reduce(am[:], tmp[:], P, ReduceOp.absmax)
            qs = qs_pool.tile([P, TSH], F32, tag="qs")
            nc.vector.tensor_scalar(
                out=qs[:], in0=am[:], scalar1=1.0 / 127.0,
                scalar2=None, op0=mybir.AluOpType.mult)
            nc.vector.reciprocal(am[:], qs[:])   # am <- 127/absmax
            nc.vector.tensor_tensor(
                out=tmp[:], in0=tmp[:], in1=am[:],
                op=mybir.AluOpType.mult)
            qi = qi_pool.tile([P, TSH], I8, tag="qi")
            nc.vector.tensor_copy(qi[:], tmp[:])
            nc.sync.dma_start(out[ts(ot, P), :], qi[:])
            nc.sync.dma_start(out_s[ot:ot + 1, :], qs[0:1, :])

    nc.compile()
    return nc


def prep_idx16(idx2):
    """Host prep: full [OUT_F, NJ] int32 -> compact wrapped int16 gather
    lists [O_TILES*JC, 16, NJC*VDIM] (no 8x replication; device broadcasts).

    Per (ot, jc): glist[g] for g = j*128 + o, wrapped[c, s] = glist[s*16+c].
    """
    A = idx2.reshape(O_TILES, P, JC, NJC)
    B = A.transpose(0, 2, 3, 1)                 # [ot, jc, j, o]
    C = B.reshape(O_TILES, JC, NJC * P)         # glist, g = j*128 + o
    D = C.reshape(O_TILES, JC, (NJC * P) // 16, 16)
    E = D.transpose(0, 1, 3, 2)                 # [ot, jc, c, s]
    return np.ascontiguousarray(E.reshape(O_TILES * JC, 16, NJC * VDIM)).astype(np.int16)


_NC_CACHE = []


def _get_nc():
    if not _NC_CACHE:
        _NC_CACHE.append(build())
    return _NC_CACHE[0]


def make_in_maps(x, indices, codebook, scales):
    x2 = np.asarray(x).reshape(T, IN_F)
    amax = np.abs(x2).max(axis=1, keepdims=True)
    s = np.where(amax > 0, amax / 127.0, 1.0).astype(np.float32)
    xq = np.clip(np.rint(x2 * (1.0 / s)), -127, 127).astype(np.int8)
    sflat = s.reshape(T)
    idx2 = np.asarray(indices, dtype=np.int32).reshape(OUT_F, NJ)
    idx16 = prep_idx16(idx2)
    sc_t = np.ascontiguousarray(
        np.asarray(scales, dtype=np.float32).reshape(O_TILES, P).T)
    cbv = np.asarray(codebook, dtype=np.float32).astype(NP_BF16)
    in_maps = []
    for c in range(N_CORES):
        in_maps.append({
            "xq": xq[c * TSH:(c + 1) * TSH],
            "xsc": np.ascontiguousarray(
                sflat[c * TSH:(c + 1) * TSH]).reshape(1, TSH),
            "cbb": cbv,
            "idx16": idx16[c * LOT * JC:(c + 1) * LOT * JC],
            "scales_t": sc_t,
        })
    return in_maps


def assemble_out(res):
    """int8 [4096 o, 512 t] + scales [32, 512] per core -> [BATCH, SEQ, OUT_F] f32."""
    out = np.empty((T, OUT_F), dtype=np.float32)
    for c in range(N_CORES):
        q = res.results[c]["out"].astype(np.float32).reshape(O_TILES, P, TSH)
        s = res.results[c]["out_s"]
        out[c * TSH:(c + 1) * TSH, :] = (q * s[:, None, :]).reshape(OUT_F, TSH).T
    return out.reshape(BATCH, SEQ, OUT_F)


def kernel(x, indices, codebook, scales):
    nc = _get_nc()
    in_maps = make_in_maps(x, indices, codebook, scales)
    res = bass_utils.run_bass_kernel_spmd(nc, in_maps, core_ids=list(range(N_CORES)))
    return assemble_out(res)


# revision 17
# speedup vs baseline: 1.1132x; 1.0023x over previous
"""HQLinear (VQ codebook) Trainium2 kernel — data-parallel tokens, int8 x,
sharded dequantize with on-device AllGather of the weight.

Computes: out = einsum('bsi,oi->bso', x, codebook[indices].reshape(O, I) * scales)
on 8 NeuronCores. The axon tunnel moves ~55-90 MB/s, so wall clock is
dominated by host->device bytes. Wire layout (~100 MB total vs ~1.6 GB for
the naive tensor-parallel layout):
  - x sharded over tokens (512/core), int8-quantized per token on host
    (45 MB); the per-token scale is applied in the f32 epilogue, so the
    int8 values flow exactly through the bf16 matmul.
  - indices sharded over out rows (512 rows/core, compact int16, 11 MB
    total): each core gathers + transposes its 1/8 of the weight, then a
    DRAM AllGather replicates the pair-packed transposed weight (the 8x
    expansion rides NeuronLink instead of the tunnel).
  - codebook uploaded pre-cast bf16 (0.5 MB/core).
  - output int8-quantized on device per (o-tile, token) block with f32
    block scales (down 17 MB, donated zeros 17 MB); host dequantizes.

Per-core pipeline:
  - codebook bf16 copied DRAM->DRAM into 256B-stride rows (dma_gather's
    source stride must be a multiple of 256B).
  - x shard [512 tok, 11008] int8 loaded in 2048-column chunks, cast
    i8->bf16 (exact), PE-transposed pair-packed (2 bf16 viewed as one f32
    lane) into a resident SBUF xT (i on partitions), 11.3 MB.
  - per local 128-out-row tile (4): DMA-gather 1376*128 codebook vectors
    (16B each) into SBUF staging [128 o, 11008 i] bf16, PE-transpose
    pair-packed to wT [i-pairs, 128 o], store to DRAM wt_loc.
  - AllGather wt_loc [4,128,5504] f32 -> wt_all [32,128,5504] (ranks
    concatenate: global o-tile ot <- core ot//4, local tile ot%4).
  - per global o-tile (32): DMA wt_all[ot] to SBUF, 86 bf16 matmuls
    (N=512 tokens) accumulate into PSUM [128 o, 512 t]; epilogue applies
    scales[o] (per-partition) then the per-token x scale (free-dim
    broadcast) in f32, computes the per-token block absmax with a gpsimd
    partition reduce, and emits int8 values + f32 scales.
  - index lists are uploaded compact ([16, 688] per gather) and replicated
    to the 8 16-row partition groups by a broadcast DMA on device.

Pair packing: an f32 lane at pair index f holds bf16 values for i = 2f,
2f+1; matmul (icp, h) contracts partitions p <-> i = 128*2*icp + 2p + h on
both operands via stride-2 bf16 views.
"""

from contextlib import ExitStack

import numpy as np

import concourse.ap_utils as ap_utils
import concourse.bass as bass
import concourse.tile as tile
from concourse import bacc, mybir
from concourse.bass import ts, ds, exact_div
from concourse.bass_isa import ReduceOp
from concourse.masks import make_identity
import concourse.bass_utils as bass_utils

F32 = mybir.dt.float32
BF16 = mybir.dt.bfloat16
I16 = mybir.dt.int16
I8 = mybir.dt.int8
P = 128

N_CORES = 8
OUT_F = 4096
IN_F = 11008
VDIM = 8
N_CODES = 32768
BATCH, SEQ = 2, 2048
T = BATCH * SEQ            # 4096 tokens
TSH = T // N_CORES         # 512 tokens per core
T_TILES = TSH // P         # 4 token tiles per core
O_TILES = OUT_F // P       # 32 out-row tiles (full weight, post-allgather)
LOT = O_TILES // N_CORES   # 4 local o-tiles gathered per core
NJ = IN_F // VDIM          # 1376 index columns per out row
JC = 16                    # gather chunks per 128-row o-tile
NJC = NJ // JC             # 86 j-columns per gather (11008 idx <= HW limit)
ICP = IN_F // 256          # 43 pair chunks (256 i-values each)
GRP = 8                    # icp per transpose/copy group (2 PSUM banks)

NP_BF16 = mybir.dt.np(BF16)


def _dma_gather_small(gp, out_ap, in_ap, idxs_ap, num_idxs, elem_size, elem_step):
    """dma_gather with small elements (16B); source stride still 256B-aligned.

    Vector g comes from in_[list[g], :elem_size] (row stride elem_step) and
    lands at out[g%128, g//128, :]. Index list int16, wrapped: idxs[c, s] =
    list[s*16 + c] for c in 0..15, replicated across the 8 16-row groups.
    """
    assert idxs_ap.dtype == I16
    assert in_ap.dtype == out_ap.dtype
    assert in_ap.space == bass.MemorySpace.DRAM
    assert idxs_ap.space == bass.MemorySpace.SBUF
    assert out_ap.space == bass.MemorySpace.SBUF
    assert ap_utils.ap_is_contiguous(in_ap.ap[1:])
    assert ap_utils.ap_is_contiguous(out_ap.ap[1:])
    assert ap_utils.ap_is_contiguous(idxs_ap.ap[1:])
    assert in_ap.ap[-1][1] == elem_size
    assert out_ap.ap[-1][1] == elem_size
    assert in_ap.ap[0][0] == elem_step
    stride_bytes_256 = exact_div(elem_step * mybir.dt.size(in_ap.dtype), 256)
    assert 0 < stride_bytes_256 < 256
    _in_ap = gp.lower_ap_dma(in_ap, for_custom_bir_dma=True)
    _idxs_ap = gp.lower_ap(idxs_ap)
    _out_ap = gp.lower_ap(out_ap)
    return gp.add_instruction(
        mybir.InstDMAGatherAnt(
            name=gp.bass.get_next_instruction_name(),
            ins=[*_in_ap, _idxs_ap, gp.lower_val_access(gp.to_reg(num_idxs))],
            outs=[_out_ap],
            transpose=False,
            num_idxs=num_idxs,
            elem_size=elem_size,
            stride_bytes_256=stride_bytes_256,
            gen_mode=0,
            single_packet=False,
            queue_num=0,
            sbuf_tokens_per_rank=0,
            sbuf_free_dim_per_rank=0,
            sbuf_free_dim_pad_per_rank=0,
            sbuf_byte_offset=0,
        )
    )


def build():
    """Build and compile the per-core kernel. Returns the Bacc instance."""
    groups = [(g, min(GRP, ICP - g)) for g in range(0, ICP, GRP)]

    nc = bacc.Bacc("TRN2", target_bir_lowering=False, debug=False,
                   enable_asserts=False, num_devices=N_CORES)

    xq = nc.dram_tensor("xq", [TSH, IN_F], I8, kind="ExternalInput").ap()
    xsc = nc.dram_tensor("xsc", [1, TSH], F32, kind="ExternalInput").ap()
    cbb = nc.dram_tensor("cbb", [N_CODES, VDIM], BF16, kind="ExternalInput").ap()
    idx16 = nc.dram_tensor("idx16", [LOT * JC, 16, NJC * VDIM], I16,
                           kind="ExternalInput").ap()
    # scales pre-transposed on host: sc_t[p, ot] = scales[ot*128 + p]
    scales_t = nc.dram_tensor("scales_t", [P, O_TILES], F32,
                              kind="ExternalInput").ap()
    out = nc.dram_tensor("out", [OUT_F, TSH], I8, kind="ExternalOutput").ap()
    out_s = nc.dram_tensor("out_s", [O_TILES, TSH], F32, kind="ExternalOutput").ap()
    cb_pad = nc.dram_tensor("cb_pad", [N_CODES, 128], BF16, kind="Internal").ap()
    wt_loc = nc.dram_tensor("wt_loc", [LOT, P, ICP * P], F32, kind="Internal").ap()
    wt_all = nc.dram_tensor("wt_all", [O_TILES, P, ICP * P], F32,
                            kind="Internal", addr_space="Shared").ap()

    with tile.TileContext(nc) as tc, ExitStack() as ctx:
        const_pool = ctx.enter_context(tc.tile_pool(name="const", bufs=1))
        xt_pool = ctx.enter_context(tc.tile_pool(name="xt", bufs=1))

        identity = const_pool.tile([P, P], F32)
        make_identity(nc, identity[:])

        sc_sb = const_pool.tile([P, O_TILES], F32)
        nc.sync.dma_start(sc_sb[:], scales_t)

        # per-token x scale, broadcast across partitions: [128, 512]
        sxt_sb = const_pool.tile([P, TSH], F32)
        nc.sync.dma_start(sxt_sb[:], xsc.to_broadcast([P, TSH]))

        # --- codebook bf16 -> padded 256B-stride rows (DRAM->DRAM) ---
        cb_pad3 = cb_pad.rearrange("(p r) c -> p r c", p=P)[:, :, :VDIM]
        nc.sync.dma_start(
            cb_pad3, cbb.rearrange("(p r) c -> p r c", p=P))

        # resident xT (pair-packed, f32-typed): free = icp*TSH + tt*128 + t
        xT = xt_pool.tile([P, ICP * TSH], F32)
        xT4 = xT[:].rearrange("p (i tt t) -> p i tt t", tt=T_TILES, t=P)

        stage_pool = ctx.enter_context(tc.tile_pool(name="stage", bufs=2))
        tpsum_pool = ctx.enter_context(
            tc.tile_pool(name="tpsum", bufs=2, space="PSUM"))
        wt_pool = ctx.enter_context(tc.tile_pool(name="wt", bufs=2))
        idx_pool = ctx.enter_context(tc.tile_pool(name="idxp", bufs=3))
        xq_pool = ctx.enter_context(tc.tile_pool(name="xq", bufs=2))
        xs_pool = ctx.enter_context(tc.tile_pool(name="xs", bufs=2))

        # --- gather + transpose local weight shard, store to wt_loc ---
        for lot in range(LOT):
            wst = stage_pool.tile([P, IN_F], BF16, tag="stage")
            # gather: wst[p, 8j:8j+8] = cbb[idx[lot*128+p, j], :]
            for jc in range(JC):
                idx_t = idx_pool.tile([P, NJC * VDIM], I16, tag="idx")
                src = idx16[lot * JC + jc, :, :]
                nc.sync.dma_start(
                    idx_t[:], src.unsqueeze(0).to_broadcast(
                        [P // 16, 16, NJC * VDIM]))
                _dma_gather_small(
                    nc.gpsimd,
                    out_ap=wst[:, jc * NJC * VDIM:(jc + 1) * NJC * VDIM]
                        .rearrange("p (n e) -> p n e", e=VDIM),
                    in_ap=cb_pad[:, :VDIM],
                    idxs_ap=idx_t[:],
                    num_idxs=NJC * P,
                    elem_size=VDIM,
                    elem_step=128,
                )
            # transpose to wT [i-pairs, 128 o] (f32 pair lanes)
            wT = wt_pool.tile([P, ICP * P], F32, tag="wt")
            wT3 = wT[:].rearrange("p (i o) -> p i o", o=P)
            wstv = wst[:].bitcast(F32)  # [128 o, 5504 pair lanes]
            for g0, glen in groups:
                tp = tpsum_pool.tile([P, GRP * P], F32, tag="tp")
                for q in range(glen):
                    nc.tensor.transpose(
                        out=tp[:, ts(q, P)],
                        in_=wstv[:, ts(g0 + q, P)],
                        identity=identity[:],
                    )
                src = tp[:, :glen * P].rearrange("p (i o) -> p i o", o=P)
                nc.vector.tensor_copy(wT3[:, ds(g0, glen), :], src)
            nc.sync.dma_start(wt_loc[lot], wT[:])

        # --- allgather the pair-packed transposed weight across cores ---
        nc.gpsimd.collective_compute(
            "AllGather",
            mybir.AluOpType.bypass,
            replica_groups=[list(range(N_CORES))],
            ins=[wt_loc[:].opt()],
            outs=[wt_all[:].opt()],
        )

        # --- dequant x (i8 -> bf16, exact) + transpose into resident xT ---
        for tt in range(T_TILES):
            for g0, glen in groups:
                ncol = glen * 256
                xq_t = xq_pool.tile([P, GRP * 128], BF16, tag="xq")
                nc.sync.dma_start(xq_t[:].bitcast(I8)[:, :ncol],
                                  xq[ts(tt, P), g0 * 256:(g0 + glen) * 256])
                xs = xs_pool.tile([P, GRP * 256], BF16, tag="xs")
                nc.vector.tensor_copy(xs[:, :ncol], xq_t[:].bitcast(I8)[:, :ncol])
                xsv = xs[:].bitcast(F32)  # [128 t, pair lanes]
                tp = tpsum_pool.tile([P, GRP * P], F32, tag="tp")
                for q in range(glen):
                    nc.tensor.transpose(
                        out=tp[:, ts(q, P)],
                        in_=xsv[:, ts(q, P)],
                        identity=identity[:],
                    )
                src = tp[:, :glen * P].rearrange("p (i t) -> p i t", t=P)
                nc.vector.tensor_copy(xT4[:, ds(g0, glen), tt, :], src)

        # bf16 view of xT: free = 2*(icp*TSH + tt*128 + t) + h
        xTb = xT[:].bitcast(BF16)

        # --- main loop over global out-row tiles ---
        opsum_pool = ctx.enter_context(
            tc.tile_pool(name="opsum", bufs=2, space="PSUM"))
        tmp_pool = ctx.enter_context(tc.tile_pool(name="tmp", bufs=2))
        am_pool = ctx.enter_context(tc.tile_pool(name="am", bufs=2))
        qs_pool = ctx.enter_context(tc.tile_pool(name="qs", bufs=2))
        qi_pool = ctx.enter_context(tc.tile_pool(name="qi", bufs=2))

        for ot in range(O_TILES):
            wT = wt_pool.tile([P, ICP * P], F32, tag="wt")
            nc.sync.dma_start(wT[:], wt_all[ot])
            wTb = wT[:].bitcast(BF16)  # free = 2*(icp*128 + o) + h

            # 86 matmuls accumulate PSUM [128 o, 512 t]
            po = opsum_pool.tile([P, TSH], F32, tag="op")
            for icp in range(ICP):
                for h in range(2):
                    lhsT = wTb[:, 2 * icp * P + h: 2 * (icp + 1) * P: 2]
                    rhs = xTb[:, 2 * icp * TSH + h: 2 * (icp + 1) * TSH: 2]
                    nc.tensor.matmul(out=po[:], lhsT=lhsT, rhs=rhs,
                                     start=(icp == 0 and h == 0),
                                     stop=(icp == ICP - 1 and h == 1))

            # epilogue: scales[o] (per-partition), then per-token x scale
            # (free-dim broadcast); int8-quantize per (o-tile, token) block
            tmp = tmp_pool.tile([P, TSH], F32, tag="tmp")
            nc.vector.tensor_scalar(
                out=tmp[:], in0=po[:], scalar1=sc_sb[:, ot:ot + 1],
                scalar2=None, op0=mybir.AluOpType.mult)
            nc.vector.tensor_tensor(
                out=tmp[:], in0=tmp[:], in1=sxt_sb[:],
                op=mybir.AluOpType.mult)
            am = am_pool.tile([P, TSH], F32, tag="am")
            nc.gpsimd.partition_all_reduce(am[:], tmp[:], P, ReduceOp.absmax)
            qs = qs_pool.tile([P, TSH], F32, tag="qs")
            nc.vector.tensor_scalar(
                out=qs[:], in0=am[:], scalar1=1.0 / 127.0,
                scalar2=None, op0=mybir.AluOpType.mult)
            nc.vector.reciprocal(am[:], qs[:])   # am <- 127/absmax
            nc.vector.tensor_tensor(
                out=tmp[:], in0=tmp[:], in1=am[:],
                op=mybir.AluOpType.mult)
            qi = qi_pool.tile([P, TSH], I8, tag="qi")
            nc.vector.tensor_copy(qi[:], tmp[:])
            nc.sync.dma_start(out[ts(ot, P), :], qi[:])
            nc.sync.dma_start(out_s[ot:ot + 1, :], qs[0:1, :])

    nc.compile()
    return nc


def prep_idx16(idx2):
    """Host prep: full [OUT_F, NJ] int32 -> compact wrapped int16 gather
    lists [O_TILES*JC, 16, NJC*VDIM] (no 8x replication; device broadcasts).

    Per (ot, jc): glist[g] for g = j*128 + o, wrapped[c, s] = glist[s*16+c].
    """
    A = idx2.reshape(O_TILES, P, JC, NJC)
    B = A.transpose(0, 2, 3, 1)                 # [ot, jc, j, o]
    C = B.reshape(O_TILES, JC, NJC * P)         # glist, g = j*128 + o
    D = C.reshape(O_TILES, JC, (NJC * P) // 16, 16)
    E = D.transpose(0, 1, 3, 2)                 # [ot, jc, c, s]
    return E.astype(np.int16, order="C").reshape(O_TILES * JC, 16, NJC * VDIM)


_NC_CACHE = []


def _get_nc():
    if not _NC_CACHE:
        _NC_CACHE.append(build())
    return _NC_CACHE[0]


def make_in_maps(x, indices, codebook, scales):
    x2 = np.asarray(x).reshape(T, IN_F)
    amax = np.abs(x2).max(axis=1, keepdims=True)
    s = np.where(amax > 0, amax / 127.0, 1.0).astype(np.float32)
    # |x2/s| <= 127 by construction, so no clip needed; rint in place
    buf = x2 * (1.0 / s)
    np.rint(buf, out=buf)
    xq = buf.astype(np.int8)
    sflat = s.reshape(T)
    idx2 = np.asarray(indices, dtype=np.int32).reshape(OUT_F, NJ)
    idx16 = prep_idx16(idx2)
    sc_t = np.ascontiguousarray(
        np.asarray(scales, dtype=np.float32).reshape(O_TILES, P).T)
    cbv = np.asarray(codebook, dtype=np.float32).astype(NP_BF16)
    in_maps = []
    for c in range(N_CORES):
        in_maps.append({
            "xq": xq[c * TSH:(c + 1) * TSH],
            "xsc": np.ascontiguousarray(
                sflat[c * TSH:(c + 1) * TSH]).reshape(1, TSH),
            "cbb": cbv,
            "idx16": idx16[c * LOT * JC:(c + 1) * LOT * JC],
            "scales_t": sc_t,
        })
    return in_maps


def assemble_out(res):
    """int8 [4096 o, 512 t] + scales [32, 512] per core -> [BATCH, SEQ, OUT_F] f32."""
    out = np.empty((T, OUT_F), dtype=np.float32)
    for c in range(N_CORES):
        q = res.results[c]["out"].astype(np.float32).reshape(O_TILES, P, TSH)
        s = res.results[c]["out_s"]
        out[c * TSH:(c + 1) * TSH, :] = (q * s[:, None, :]).reshape(OUT_F, TSH).T
    return out.reshape(BATCH, SEQ, OUT_F)


def kernel(x, indices, codebook, scales):
    nc = _get_nc()
    in_maps = make_in_maps(x, indices, codebook, scales)
    res = bass_utils.run_bass_kernel_spmd(nc, in_maps, core_ids=list(range(N_CORES)))
    return assemble_out(res)


# revision 18
# speedup vs baseline: 1.1409x; 1.0249x over previous
"""HQLinear (VQ codebook) Trainium2 kernel — data-parallel tokens, int8 x,
sharded dequantize with on-device AllGather of the weight.

Computes: out = einsum('bsi,oi->bso', x, codebook[indices].reshape(O, I) * scales)
on 8 NeuronCores. The axon tunnel moves ~55-90 MB/s, so wall clock is
dominated by host->device bytes. Wire layout (~100 MB total vs ~1.6 GB for
the naive tensor-parallel layout):
  - x sharded over tokens (512/core), int8-quantized per token on host
    (45 MB); the per-token scale is applied in the f32 epilogue, so the
    int8 values flow exactly through the bf16 matmul.
  - indices sharded over out rows (512 rows/core, compact int16, 11 MB
    total): each core gathers + transposes its 1/8 of the weight, then a
    DRAM AllGather replicates the pair-packed transposed weight (the 8x
    expansion rides NeuronLink instead of the tunnel).
  - codebook uploaded pre-cast bf16 (0.5 MB/core).
  - output int8-quantized on device per (o-tile, token) block with f32
    block scales (down 17 MB, donated zeros 17 MB); host dequantizes.

Per-core pipeline:
  - codebook bf16 copied DRAM->DRAM into 256B-stride rows (dma_gather's
    source stride must be a multiple of 256B).
  - x shard [512 tok, 11008] int8 loaded in 2048-column chunks, cast
    i8->bf16 (exact), PE-transposed pair-packed (2 bf16 viewed as one f32
    lane) into a resident SBUF xT (i on partitions), 11.3 MB.
  - per local 128-out-row tile (4): DMA-gather 1376*128 codebook vectors
    (16B each) into SBUF staging [128 o, 11008 i] bf16, PE-transpose
    pair-packed to wT [i-pairs, 128 o], store to DRAM wt_loc.
  - AllGather wt_loc [4,128,5504] f32 -> wt_all [32,128,5504] (ranks
    concatenate: global o-tile ot <- core ot//4, local tile ot%4).
  - per global o-tile (32): DMA wt_all[ot] to SBUF, 86 bf16 matmuls
    (N=512 tokens) accumulate into PSUM [128 o, 512 t]; epilogue applies
    scales[o] (per-partition) then the per-token x scale (free-dim
    broadcast) in f32, computes the per-token block absmax with a gpsimd
    partition reduce, and emits int8 values + f32 scales.
  - index lists are uploaded compact ([16, 688] per gather) and replicated
    to the 8 16-row partition groups by a broadcast DMA on device.

Pair packing: an f32 lane at pair index f holds bf16 values for i = 2f,
2f+1; matmul (icp, h) contracts partitions p <-> i = 128*2*icp + 2p + h on
both operands via stride-2 bf16 views.
"""

from contextlib import ExitStack

import numpy as np

import concourse.ap_utils as ap_utils
import concourse.bass as bass
import concourse.tile as tile
from concourse import bacc, mybir
from concourse.bass import ts, ds, exact_div
from concourse.bass_isa import ReduceOp
from concourse.masks import make_identity
import concourse.bass_utils as bass_utils

F32 = mybir.dt.float32
BF16 = mybir.dt.bfloat16
I16 = mybir.dt.int16
I8 = mybir.dt.int8
P = 128

N_CORES = 8
OUT_F = 4096
IN_F = 11008
VDIM = 8
N_CODES = 32768
BATCH, SEQ = 2, 2048
T = BATCH * SEQ            # 4096 tokens
TSH = T // N_CORES         # 512 tokens per core
T_TILES = TSH // P         # 4 token tiles per core
O_TILES = OUT_F // P       # 32 out-row tiles (full weight, post-allgather)
LOT = O_TILES // N_CORES   # 4 local o-tiles gathered per core
NJ = IN_F // VDIM          # 1376 index columns per out row
JC = 16                    # gather chunks per 128-row o-tile
NJC = NJ // JC             # 86 j-columns per gather (11008 idx <= HW limit)
ICP = IN_F // 256          # 43 pair chunks (256 i-values each)
GRP = 8                    # icp per transpose/copy group (2 PSUM banks)

NP_BF16 = mybir.dt.np(BF16)


def _dma_gather_small(gp, out_ap, in_ap, idxs_ap, num_idxs, elem_size, elem_step):
    """dma_gather with small elements (16B); source stride still 256B-aligned.

    Vector g comes from in_[list[g], :elem_size] (row stride elem_step) and
    lands at out[g%128, g//128, :]. Index list int16, wrapped: idxs[c, s] =
    list[s*16 + c] for c in 0..15, replicated across the 8 16-row groups.
    """
    assert idxs_ap.dtype == I16
    assert in_ap.dtype == out_ap.dtype
    assert in_ap.space == bass.MemorySpace.DRAM
    assert idxs_ap.space == bass.MemorySpace.SBUF
    assert out_ap.space == bass.MemorySpace.SBUF
    assert ap_utils.ap_is_contiguous(in_ap.ap[1:])
    assert ap_utils.ap_is_contiguous(out_ap.ap[1:])
    assert ap_utils.ap_is_contiguous(idxs_ap.ap[1:])
    assert in_ap.ap[-1][1] == elem_size
    assert out_ap.ap[-1][1] == elem_size
    assert in_ap.ap[0][0] == elem_step
    stride_bytes_256 = exact_div(elem_step * mybir.dt.size(in_ap.dtype), 256)
    assert 0 < stride_bytes_256 < 256
    _in_ap = gp.lower_ap_dma(in_ap, for_custom_bir_dma=True)
    _idxs_ap = gp.lower_ap(idxs_ap)
    _out_ap = gp.lower_ap(out_ap)
    return gp.add_instruction(
        mybir.InstDMAGatherAnt(
            name=gp.bass.get_next_instruction_name(),
            ins=[*_in_ap, _idxs_ap, gp.lower_val_access(gp.to_reg(num_idxs))],
            outs=[_out_ap],
            transpose=False,
            num_idxs=num_idxs,
            elem_size=elem_size,
            stride_bytes_256=stride_bytes_256,
            gen_mode=0,
            single_packet=False,
            queue_num=0,
            sbuf_tokens_per_rank=0,
            sbuf_free_dim_per_rank=0,
            sbuf_free_dim_pad_per_rank=0,
            sbuf_byte_offset=0,
        )
    )


def build():
    """Build and compile the per-core kernel. Returns the Bacc instance."""
    groups = [(g, min(GRP, ICP - g)) for g in range(0, ICP, GRP)]

    nc = bacc.Bacc("TRN2", target_bir_lowering=False, debug=False,
                   enable_asserts=False, num_devices=N_CORES)

    xq = nc.dram_tensor("xq", [TSH, IN_F], I8, kind="ExternalInput").ap()
    xsc = nc.dram_tensor("xsc", [1, TSH], F32, kind="ExternalInput").ap()
    cbb = nc.dram_tensor("cbb", [N_CODES, VDIM], BF16, kind="ExternalInput").ap()
    idx16 = nc.dram_tensor("idx16", [LOT * JC, 16, NJC * VDIM], I16,
                           kind="ExternalInput").ap()
    # scales pre-transposed on host: sc_t[p, ot] = scales[ot*128 + p]
    scales_t = nc.dram_tensor("scales_t", [P, O_TILES], F32,
                              kind="ExternalInput").ap()
    out = nc.dram_tensor("out", [OUT_F, TSH], I8, kind="ExternalOutput").ap()
    out_s = nc.dram_tensor("out_s", [O_TILES, TSH], F32, kind="ExternalOutput").ap()
    cb_pad = nc.dram_tensor("cb_pad", [N_CODES, 128], BF16, kind="Internal").ap()
    wt_loc = nc.dram_tensor("wt_loc", [LOT, P, ICP * P], F32, kind="Internal").ap()
    wt_all = nc.dram_tensor("wt_all", [O_TILES, P, ICP * P], F32,
                            kind="Internal", addr_space="Shared").ap()

    with tile.TileContext(nc) as tc, ExitStack() as ctx:
        const_pool = ctx.enter_context(tc.tile_pool(name="const", bufs=1))
        xt_pool = ctx.enter_context(tc.tile_pool(name="xt", bufs=1))

        identity = const_pool.tile([P, P], F32)
        make_identity(nc, identity[:])

        sc_sb = const_pool.tile([P, O_TILES], F32)
        nc.sync.dma_start(sc_sb[:], scales_t)

        # per-token x scale, broadcast across partitions: [128, 512]
        sxt_sb = const_pool.tile([P, TSH], F32)
        nc.sync.dma_start(sxt_sb[:], xsc.to_broadcast([P, TSH]))

        # --- codebook bf16 -> padded 256B-stride rows (DRAM->DRAM) ---
        cb_pad3 = cb_pad.rearrange("(p r) c -> p r c", p=P)[:, :, :VDIM]
        nc.sync.dma_start(
            cb_pad3, cbb.rearrange("(p r) c -> p r c", p=P))

        # resident xT (pair-packed, f32-typed): free = icp*TSH + tt*128 + t
        xT = xt_pool.tile([P, ICP * TSH], F32)
        xT4 = xT[:].rearrange("p (i tt t) -> p i tt t", tt=T_TILES, t=P)

        stage_pool = ctx.enter_context(tc.tile_pool(name="stage", bufs=2))
        tpsum_pool = ctx.enter_context(
            tc.tile_pool(name="tpsum", bufs=2, space="PSUM"))
        wt_pool = ctx.enter_context(tc.tile_pool(name="wt", bufs=2))
        idx_pool = ctx.enter_context(tc.tile_pool(name="idxp", bufs=3))
        xq_pool = ctx.enter_context(tc.tile_pool(name="xq", bufs=2))
        xs_pool = ctx.enter_context(tc.tile_pool(name="xs", bufs=2))

        # --- gather + transpose local weight shard, store to wt_loc ---
        for lot in range(LOT):
            wst = stage_pool.tile([P, IN_F], BF16, tag="stage")
            # gather: wst[p, 8j:8j+8] = cbb[idx[lot*128+p, j], :]
            for jc in range(JC):
                idx_t = idx_pool.tile([P, NJC * VDIM], I16, tag="idx")
                src = idx16[lot * JC + jc, :, :]
                nc.sync.dma_start(
                    idx_t[:], src.unsqueeze(0).to_broadcast(
                        [P // 16, 16, NJC * VDIM]))
                _dma_gather_small(
                    nc.gpsimd,
                    out_ap=wst[:, jc * NJC * VDIM:(jc + 1) * NJC * VDIM]
                        .rearrange("p (n e) -> p n e", e=VDIM),
                    in_ap=cb_pad[:, :VDIM],
                    idxs_ap=idx_t[:],
                    num_idxs=NJC * P,
                    elem_size=VDIM,
                    elem_step=128,
                )
            # transpose to wT [i-pairs, 128 o] (f32 pair lanes)
            wT = wt_pool.tile([P, ICP * P], F32, tag="wt")
            wT3 = wT[:].rearrange("p (i o) -> p i o", o=P)
            wstv = wst[:].bitcast(F32)  # [128 o, 5504 pair lanes]
            for g0, glen in groups:
                tp = tpsum_pool.tile([P, GRP * P], F32, tag="tp")
                for q in range(glen):
                    nc.tensor.transpose(
                        out=tp[:, ts(q, P)],
                        in_=wstv[:, ts(g0 + q, P)],
                        identity=identity[:],
                    )
                src = tp[:, :glen * P].rearrange("p (i o) -> p i o", o=P)
                nc.vector.tensor_copy(wT3[:, ds(g0, glen), :], src)
            nc.sync.dma_start(wt_loc[lot], wT[:])

        # --- allgather the pair-packed transposed weight across cores ---
        nc.gpsimd.collective_compute(
            "AllGather",
            mybir.AluOpType.bypass,
            replica_groups=[list(range(N_CORES))],
            ins=[wt_loc[:].opt()],
            outs=[wt_all[:].opt()],
        )

        # --- dequant x (i8 -> bf16, exact) + transpose into resident xT ---
        for tt in range(T_TILES):
            for g0, glen in groups:
                ncol = glen * 256
                xq_t = xq_pool.tile([P, GRP * 128], BF16, tag="xq")
                nc.sync.dma_start(xq_t[:].bitcast(I8)[:, :ncol],
                                  xq[ts(tt, P), g0 * 256:(g0 + glen) * 256])
                xs = xs_pool.tile([P, GRP * 256], BF16, tag="xs")
                nc.vector.tensor_copy(xs[:, :ncol], xq_t[:].bitcast(I8)[:, :ncol])
                xsv = xs[:].bitcast(F32)  # [128 t, pair lanes]
                tp = tpsum_pool.tile([P, GRP * P], F32, tag="tp")
                for q in range(glen):
                    nc.tensor.transpose(
                        out=tp[:, ts(q, P)],
                        in_=xsv[:, ts(q, P)],
                        identity=identity[:],
                    )
                src = tp[:, :glen * P].rearrange("p (i t) -> p i t", t=P)
                nc.vector.tensor_copy(xT4[:, ds(g0, glen), tt, :], src)

        # bf16 view of xT: free = 2*(icp*TSH + tt*128 + t) + h
        xTb = xT[:].bitcast(BF16)

        # --- main loop over global out-row tiles ---
        opsum_pool = ctx.enter_context(
            tc.tile_pool(name="opsum", bufs=2, space="PSUM"))
        tmp_pool = ctx.enter_context(tc.tile_pool(name="tmp", bufs=2))
        am_pool = ctx.enter_context(tc.tile_pool(name="am", bufs=2))
        qs_pool = ctx.enter_context(tc.tile_pool(name="qs", bufs=2))
        qi_pool = ctx.enter_context(tc.tile_pool(name="qi", bufs=2))

        for ot in range(O_TILES):
            wT = wt_pool.tile([P, ICP * P], F32, tag="wt")
            nc.sync.dma_start(wT[:], wt_all[ot])
            wTb = wT[:].bitcast(BF16)  # free = 2*(icp*128 + o) + h

            # 86 matmuls accumulate PSUM [128 o, 512 t]
            po = opsum_pool.tile([P, TSH], F32, tag="op")
            for icp in range(ICP):
                for h in range(2):
                    lhsT = wTb[:, 2 * icp * P + h: 2 * (icp + 1) * P: 2]
                    rhs = xTb[:, 2 * icp * TSH + h: 2 * (icp + 1) * TSH: 2]
                    nc.tensor.matmul(out=po[:], lhsT=lhsT, rhs=rhs,
                                     start=(icp == 0 and h == 0),
                                     stop=(icp == ICP - 1 and h == 1))

            # epilogue: scales[o] (per-partition), then per-token x scale
            # (free-dim broadcast); int8-quantize per (o-tile, token) block
            tmp = tmp_pool.tile([P, TSH], F32, tag="tmp")
            nc.vector.tensor_scalar(
                out=tmp[:], in0=po[:], scalar1=sc_sb[:, ot:ot + 1],
                scalar2=None, op0=mybir.AluOpType.mult)
            nc.vector.tensor_tensor(
                out=tmp[:], in0=tmp[:], in1=sxt_sb[:],
                op=mybir.AluOpType.mult)
            am = am_pool.tile([P, TSH], F32, tag="am")
            nc.gpsimd.partition_all_reduce(am[:], tmp[:], P, ReduceOp.absmax)
            qs = qs_pool.tile([P, TSH], F32, tag="qs")
            nc.vector.tensor_scalar(
                out=qs[:], in0=am[:], scalar1=1.0 / 127.0,
                scalar2=None, op0=mybir.AluOpType.mult)
            nc.vector.reciprocal(am[:], qs[:])   # am <- 127/absmax
            nc.vector.tensor_tensor(
                out=tmp[:], in0=tmp[:], in1=am[:],
                op=mybir.AluOpType.mult)
            qi = qi_pool.tile([P, TSH], I8, tag="qi")
            nc.vector.tensor_copy(qi[:], tmp[:])
            nc.sync.dma_start(out[ts(ot, P), :], qi[:])
            nc.sync.dma_start(out_s[ot:ot + 1, :], qs[0:1, :])

    nc.compile()
    return nc


def prep_idx16(idx2):
    """Host prep: full [OUT_F, NJ] int32 -> compact wrapped int16 gather
    lists [O_TILES*JC, 16, NJC*VDIM] (no 8x replication; device broadcasts).

    Per (ot, jc): glist[g] for g = j*128 + o, wrapped[c, s] = glist[s*16+c].
    """
    A = idx2.reshape(O_TILES, P, JC, NJC)
    B = A.transpose(0, 2, 3, 1)                 # [ot, jc, j, o]
    C = B.reshape(O_TILES, JC, NJC * P)         # glist, g = j*128 + o
    D = C.reshape(O_TILES, JC, (NJC * P) // 16, 16)
    E = D.transpose(0, 1, 3, 2)                 # [ot, jc, c, s]
    return E.astype(np.int16, order="C").reshape(O_TILES * JC, 16, NJC * VDIM)


_NC_CACHE = []


def _get_nc():
    if not _NC_CACHE:
        _NC_CACHE.append(build())
    return _NC_CACHE[0]


def make_in_maps(x, indices, codebook, scales):
    x2 = np.asarray(x).reshape(T, IN_F)
    amax = np.abs(x2).max(axis=1, keepdims=True)
    s = np.where(amax > 0, amax / 127.0, 1.0).astype(np.float32)
    # |x2/s| <= 127 by construction, so no clip needed; rint in place
    buf = x2 * (1.0 / s)
    np.rint(buf, out=buf)
    xq = buf.astype(np.int8)
    sflat = s.reshape(T)
    idx2 = np.asarray(indices, dtype=np.int32).reshape(OUT_F, NJ)
    idx16 = prep_idx16(idx2)
    sc_t = np.ascontiguousarray(
        np.asarray(scales, dtype=np.float32).reshape(O_TILES, P).T)
    cbv = np.asarray(codebook, dtype=np.float32).astype(NP_BF16)
    in_maps = []
    for c in range(N_CORES):
        in_maps.append({
            "xq": xq[c * TSH:(c + 1) * TSH],
            "xsc": np.ascontiguousarray(
                sflat[c * TSH:(c + 1) * TSH]).reshape(1, TSH),
            "cbb": cbv,
            "idx16": idx16[c * LOT * JC:(c + 1) * LOT * JC],
            "scales_t": sc_t,
        })
    return in_maps


def assemble_out(res):
    """int8 [4096 o, 512 t] + scales [32, 512] per core -> [BATCH, SEQ, OUT_F] f32."""
    out = np.empty((T, OUT_F), dtype=np.float32)
    for c in range(N_CORES):
        q = res.results[c]["out"].reshape(O_TILES, P, TSH)
        s = res.results[c]["out_s"]
        out[c * TSH:(c + 1) * TSH, :] = (q * s[:, None, :]).reshape(OUT_F, TSH).T
    return out.reshape(BATCH, SEQ, OUT_F)


def kernel(x, indices, codebook, scales):
    nc = _get_nc()
    in_maps = make_in_maps(x, indices, codebook, scales)
    res = bass_utils.run_bass_kernel_spmd(nc, in_maps, core_ids=list(range(N_CORES)))
    return assemble_out(res)
